# revision 2
# baseline (speedup 1.0000x reference)
"""CIN kernel v3 for Trainium2, 8 NeuronCores — column-pipelined.

Problem: x [4096, 39, 16]; 3 CIN layers (size 128):
  out_k[b,s,d] = sum_{i,j} x[b,i,d] * prev[b,j,d] * w_k[i*Fk+j, s] + b_k[s]
Output: sum_d concat(out_1, out_2) -> [4096, 256]  (layer-0 output dropped).

Design (data-parallel, 512 rows/core, bd=(b,d) b-major, 8192 cols/core):
  The whole pipeline is column-blocked (CB=1024): L0 -> L1 -> Gram -> final
  per block, so DMA / PE / DVE / ACT overlap across blocks.
  L0: banded-symmetric pairs (10 chunks, rows (u,t) = pair (i0+u, i0+u+t)),
      z0 = in0z (broadcast of x_i rows) * in1z (banded window rows), both
      loaded by DMA; 128x512 matmuls accumulate over chunks.
  L1: z1_i = out0c * bcast(x_i).  Broadcast tiles come from two generators
      running concurrently: PE selector-transposes (one-hot lhsT patterns,
      bf16 PSUM output keeps the DVE multiply at 2x rate) and DMA
      partition-broadcast quads — the split is tuned so neither engine
      saturates.
  Layer 2 never materializes out2: sum_d out2[b,:,d] = w2^T.vec(G2_b)+16*b2
      with per-sample Gram G2_b computed by PE-transposing out1 column
      blocks (8 samples each) against a host-built block-diagonal probe.
  Final contraction accumulates per half (N=256) into one PSUM bank.
"""
import sys

for p in ("/opt/trn_rl_repo",):
    if p not in sys.path:
        sys.path.insert(0, p)

import contextlib

import numpy as np
import ml_dtypes

import concourse.bass as bass
import concourse.mybir as mybir
import concourse.tile as tile
from concourse import bacc
from concourse.bass_utils import run_bass_kernel_spmd
from concourse.masks import make_identity

F32 = mybir.dt.float32
BF16 = mybir.dt.bfloat16

N_CORES = 8
B, F0, D = 4096, 39, 16
S = 128
BC = B // N_CORES            # 512
BD = BC * D                  # 8192
CB = 1024                    # column block
NCB = BD // CB               # 8
M1 = F0 * S                  # 4992
XTP = 128                    # padded xT rows (full PE partition height)

# layer-0 banded-symmetric chunks: (i0, L, k)
L0_CHUNKS = [
    (0, 39, 3), (3, 39, 3), (6, 39, 3), (9, 39, 3),
    (12, 27, 4), (16, 27, 4), (20, 27, 4), (24, 27, 2),
    (26, 13, 9), (35, 13, 9),
]
NC0 = len(L0_CHUNKS)         # 10

GB = 8                       # samples per Gram group
GN = GB * F0                 # 312
NGRP = BC // GB              # 64 groups
GPC = CB // (GB * D)         # 8 groups per column block

# L1 broadcast generator per i: "pef" = PE fp32 selector-matmul + ACT copy,
# rest = DMA partition-bcast quads (gps = gpsimd partition_broadcast failed
# to compile; dropped).  Tuned so PE / ACT / DMA loads balance.
PE_BCAST = set(range(16, 39))
GPS_BCAST = set()
DMA_I = [i for i in range(F0) if i not in PE_BCAST and i not in GPS_BCAST]
# i-loop processing order interleaves DMA- and PE-sourced tiles so neither
# engine sees a long valley within a column block (accumulation over i is
# order-independent).
PEF_I = sorted(PE_BCAST)
I_ORDER = []
_a, _b = 0, 0
for _n in range(F0):
    # ratio ~22 DMA : 17 pef
    if (_a * len(PEF_I) <= _b * len(DMA_I) or _b >= len(PEF_I)) \
            and _a < len(DMA_I):
        I_ORDER.append(DMA_I[_a])
        _a += 1
    else:
        I_ORDER.append(PEF_I[_b])
        _b += 1

# single-DMA banded loads for L0 (HW-validated with flat out APs)
FANCY_L0_DMA = True


def build_program():
    nc = bacc.Bacc("TRN2", target_bir_lowering=False, debug=False,
                   num_devices=N_CORES)
    t = {}
    t["xTp"] = nc.dram_tensor("xTp", [XTP, BD], BF16, kind="ExternalInput")
    t["w0sb"] = nc.dram_tensor("w0sb", [117, NC0 * S], BF16,
                               kind="ExternalInput")
    t["w1sb"] = nc.dram_tensor("w1sb", [S, M1], BF16, kind="ExternalInput")
    t["w2sb"] = nc.dram_tensor("w2sb", [S, M1], BF16, kind="ExternalInput")
    t["xbdh0"] = nc.dram_tensor("xbdh0", [128, NGRP * GN // 2], BF16,
                                kind="ExternalInput")
    t["xbdh1"] = nc.dram_tensor("xbdh1", [128, NGRP * GN // 2], BF16,
                                kind="ExternalInput")
    t["selp"] = nc.dram_tensor("selp", [128, F0 * S], BF16,
                               kind="ExternalInput")
    t["b0"] = nc.dram_tensor("b0", [S, 1], F32, kind="ExternalInput")
    t["b1"] = nc.dram_tensor("b1", [S, 1], F32, kind="ExternalInput")
    t["b1x"] = nc.dram_tensor("b1x", [S, 1], F32, kind="ExternalInput")
    t["b2x"] = nc.dram_tensor("b2x", [S, 1], F32, kind="ExternalInput")
    t["out"] = nc.dram_tensor("out", [BC, 2 * S], F32, kind="ExternalOutput")

    with tile.TileContext(nc) as tc:
        _body(nc, tc, t)
    nc.compile()
    return nc


def _body(nc, tc, t):
    MUL = mybir.AluOpType.mult
    ADD = mybir.AluOpType.add
    IDENT = mybir.ActivationFunctionType.Identity
    COPY = mybir.ActivationFunctionType.Copy
    AX = mybir.AxisListType.X

    xTp = t["xTp"].ap()

    ctx = contextlib.ExitStack()
    with ctx:
        const = ctx.enter_context(tc.tile_pool(name="const", bufs=1))
        acts = ctx.enter_context(tc.tile_pool(name="acts", bufs=1))
        wk = ctx.enter_context(tc.tile_pool(name="wk", bufs=1))
        ps = ctx.enter_context(tc.tile_pool(name="ps", bufs=1, space="PSUM"))

        # ---- constants ----
        b0t = const.tile([S, 1], F32, tag="b0")
        b1t = const.tile([S, 1], F32, tag="b1")
        b1xt = const.tile([S, 1], F32, tag="b1x")
        b2xt = const.tile([S, 1], F32, tag="b2x")
        nc.scalar.dma_start(out=b0t[:], in_=t["b0"].ap()[:])
        nc.scalar.dma_start(out=b1t[:], in_=t["b1"].ap()[:])
        nc.scalar.dma_start(out=b1xt[:], in_=t["b1x"].ap()[:])
        nc.scalar.dma_start(out=b2xt[:], in_=t["b2x"].ap()[:])

        identb = const.tile([128, 128], BF16, tag="identb")
        identf = const.tile([128, 128], F32, tag="identf")
        make_identity(nc, identb[:])
        make_identity(nc, identf[:])

        # load order matters: w0sb/selp/w1sb gate the first column block
        w0sb = const.tile([117, NC0 * S], BF16, tag="w0")
        nc.scalar.dma_start(out=w0sb[:], in_=t["w0sb"].ap()[:])
        selp = const.tile([128, F0 * S], BF16, tag="selp")
        nc.scalar.dma_start(out=selp[:], in_=t["selp"].ap()[:])
        w1sb = const.tile([S, M1], BF16, tag="w1")
        nc.scalar.dma_start(out=w1sb[:], in_=t["w1sb"].ap()[:])
        w2sb = const.tile([S, M1], BF16, tag="w2")
        nc.scalar.dma_start(out=w2sb[:], in_=t["w2sb"].ap()[:])
        xbdh = acts.tile([128, NGRP * GN // 2], BF16, tag="xbdh")
        nc.scalar.dma_start(out=xbdh[:], in_=t["xbdh0"].ap()[:])
        xsb = const.tile([XTP, BD], BF16, tag="xsb")
        nc.sync.dma_start(out=xsb[:], in_=xTp[:])

        outF1 = acts.tile([S, BC], F32, tag="outF1")
        outF2 = acts.tile([S, BC], F32, tag="outF2")
        g2s = acts.tile([S, NGRP * GN], BF16, tag="g2s")

        def _gram_block(item):
            gcb, g_out1c = item
            if gcb == NCB // 2:
                # reload the probe for the second half (WAR on half-0
                # gram reads is auto-tracked)
                nc.scalar.dma_start(out=xbdh[:], in_=t["xbdh1"].ap()[:])
            for l in range(GPC):
                gh = (gcb % (NCB // 2)) * GPC + l  # group index in half
                tps = ps.tile([128, 128], BF16, tag="gp", bufs=2,
                              name="tps")
                nc.tensor.transpose(tps[:],
                                    g_out1c[:, 128 * l:128 * (l + 1)],
                                    identb[:])
                o1t = wk.tile([128, 128], BF16, tag="o1t", bufs=4,
                              name="o1t")
                nc.scalar.activation(o1t[:], tps[:], COPY)
                g2p = ps.tile([128, GN], F32, tag="gp", bufs=2, name="g2p")
                nc.tensor.matmul(g2p[:], o1t[:],
                                 xbdh[:, GN * gh:GN * (gh + 1)],
                                 start=True, stop=True)
                # scatter into i-major g2s (per half) so the final matmul
                # reads a CONTIGUOUS rhs (strided rhs streams ~5x slower
                # on PE); on DVE to keep scalar off the critical path
                dst = (g2s[:, (NGRP * GN // 2) * (gcb // (NCB // 2)):]
                       [:, :NGRP * GN // 2]
                       .rearrange("p (i g kk) -> p i g kk", g=32,
                                  kk=GB)[:, :, gh, :])
                nc.vector.tensor_copy(
                    dst, g2p[:].rearrange("p (kk i) -> p i kk", kk=GB))
            if gcb % (NCB // 2) == NCB // 2 - 1:
                h = gcb // (NCB // 2)
                facc = ps.tile([S, 256], F32, tag="gp", bufs=2,
                               name=f"facc{h}")
                gof = (NGRP * GN // 2) * h
                for i in range(F0):
                    nc.tensor.matmul(
                        facc[:], w2sb[:, S * i:S * (i + 1)],
                        g2s[:, gof + 256 * i:gof + 256 * (i + 1)],
                        start=(i == 0), stop=(i == F0 - 1))
                nc.scalar.activation(outF2[:, 256 * h:256 * (h + 1)],
                                     facc[:], IDENT, bias=b2xt[:],
                                     scale=1.0)

        def _l0_block(cb):
            """Layer 0 for column block cb.  Flat [kL, CB] out zips
            against the 3-dim sources by element order (multi-dim SBUF
            partition APs do not exist; this form is HW-validated).
            Both banded loads ride the gpsimd queue (SWDGE descriptor
            generation is ~2.5x cheaper than sync HWDGE for these
            many-run APs); the sync queue carries the L1 quads."""
            csl = slice(cb * CB, (cb + 1) * CB)
            c0 = cb * CB
            acc0 = ps.tile([S, CB], F32, tag="acc0", name="acc0")
            for c, (i0, L, k) in enumerate(L0_CHUNKS):
                n = k * L
                in0z = wk.tile([117, CB], BF16, tag="in0z", bufs=6,
                               name="in0z")
                in1z = wk.tile([117, CB], BF16, tag="in1z", bufs=6,
                               name="in1z")
                nc.gpsimd.dma_start(
                    out=in0z[:n],
                    in_=xTp[i0:i0 + k, csl].unsqueeze(1)
                    .broadcast_to([k, L, CB]))
                win = bass.AP(t["xTp"], i0 * BD + c0,
                              [[BD, k], [BD, L], [1, CB]])
                nc.gpsimd.dma_start(out=in1z[:n], in_=win)
                z0 = wk.tile([117, CB], BF16, tag="z0", bufs=6, name="z0")
                # (gpsimd TT measured 2.75us/tile AND knocks DVE off its
                # 2x mode via SBUF port contention — multiplies stay on DVE)
                nc.vector.tensor_tensor(out=z0[:n, :], in0=in1z[:n, :],
                                        in1=in0z[:n, :], op=MUL)
                lhsT = w0sb[:n, S * c:S * (c + 1)]
                for g in range(CB // 512):
                    nc.tensor.matmul(acc0[:, 512 * g:512 * (g + 1)], lhsT,
                                     z0[:n, 512 * g:512 * (g + 1)],
                                     start=(c == 0), stop=(c == NC0 - 1))
            return acc0

        def _l1_block(cb, acc0):
            csl = slice(cb * CB, (cb + 1) * CB)
            c0 = cb * CB
            out0c = wk.tile([S, CB], BF16, tag="out0c", bufs=2,
                            name="out0c")
            nc.scalar.activation(out0c[:], acc0[:], IDENT, bias=b0t[:],
                                 scale=1.0)
            acc1 = ps.tile([S, CB], F32, tag="acc1", name="acc1")
            bcq = {}
            for qi, i in enumerate(DMA_I):
                if qi % 4 == 0:
                    nq = min(4, len(DMA_I) - qi)
                    bc4 = wk.tile([128, 4 * CB], BF16, tag="bc4", bufs=3,
                                  name="bc4")
                    rows = DMA_I[qi:qi + nq]
                    # rows are consecutive by construction of DMA_I
                    nc.sync.dma_start(
                        out=bc4[:, 0:nq * CB]
                        .rearrange("p (i n) -> p i n", i=nq),
                        in_=xTp[rows[0]:rows[0] + nq, csl]
                        .unsqueeze(0).partition_broadcast(128))
                    bcq[qi] = bc4
            for idx, i in enumerate(I_ORDER):
                if i in PE_BCAST:
                    bcp = wk.tile([128, CB], BF16, tag="bcp", bufs=4,
                                  name="bcp")
                    for h in range(CB // 512):
                        pzf = ps.tile([128, 512], F32, tag="pzf", bufs=2,
                                      name="pzf")
                        # K padded to 128: full-height moving operand
                        # streams at 1 col/cycle (K=39/64 pay ~1.5x)
                        nc.tensor.matmul(
                            pzf[:],
                            selp[:, S * i:S * (i + 1)],
                            xsb[:, c0 + 512 * h:c0 + 512 * (h + 1)],
                            start=True, stop=True)
                        nc.scalar.activation(
                            bcp[:, 512 * h:512 * (h + 1)], pzf[:], COPY)
                    bcap = bcp[:]
                else:
                    qi = DMA_I.index(i)
                    bc4 = bcq[qi - qi % 4]
                    u = qi % 4
                    bcap = bc4[:, u * CB:(u + 1) * CB]
                z1 = wk.tile([128, CB], BF16, tag="z1", bufs=6, name="z1")
                nc.vector.tensor_tensor(out=z1[:], in0=out0c[:], in1=bcap,
                                        op=MUL)
                lhsT = w1sb[:, S * i:S * (i + 1)]
                for g in range(CB // 512):
                    nc.tensor.matmul(acc1[:, 512 * g:512 * (g + 1)], lhsT,
                                     z1[:, 512 * g:512 * (g + 1)],
                                     start=(idx == 0), stop=(idx == F0 - 1))
            out1c = wk.tile([S, CB], BF16, tag="out1c", bufs=3,
                            name="out1c")
            nc.scalar.activation(out1c[:], acc1[:], IDENT, bias=b1t[:],
                                 scale=1.0)
            # d-sum from the fp32 accumulator (bf16 source costs ~3x in
            # output accuracy: rel err 0.005 -> 0.015)
            nc.vector.tensor_reduce(
                out=outF1[:, cb * (CB // D):(cb + 1) * (CB // D)],
                in_=acc1[:].rearrange("p (b d) -> p b d", d=D),
                axis=AX, op=ADD)
            return out1c

        # ====== rotated software pipeline ======
        # iteration j emits: ACT(out0c_j) -> L1(j) -> L0(j+1) -> gram(j-1)
        # so every engine's in-order queue has ~one block of slack for
        # every cross-engine dependency.
        pend = []
        acc0_cur = _l0_block(0)
        for cb in range(NCB):
            out1c = _l1_block(cb, acc0_cur)
            if cb + 1 < NCB:
                acc0_cur = _l0_block(cb + 1)
            pend.append((cb, out1c))
            if cb > 0:
                _gram_block(pend.pop(0))
        _gram_block(pend.pop(0))

        # ================= output =================
        nc.vector.tensor_scalar_add(outF1[:], outF1[:], b1xt[:])
        for tt in range(BC // 128):
            csl = slice(128 * tt, 128 * (tt + 1))
            otile = wk.tile([128, 2 * S], F32, tag="otile", bufs=2,
                            name="otile")
            p1 = ps.tile([128, 128], F32, tag="gp", bufs=2, name="tr1")
            nc.tensor.transpose(p1[:], outF1[:, csl], identf[:])
            nc.vector.tensor_copy(otile[:, 0:S], p1[:])
            p2 = ps.tile([128, 128], F32, tag="gp", bufs=2, name="tr2")
            nc.tensor.transpose(p2[:], outF2[:, csl], identf[:])
            nc.vector.tensor_copy(otile[:, S:2 * S], p2[:])
            nc.sync.dma_start(out=t["out"].ap()[csl, :], in_=otile[:])


_PROGRAM_CACHE = {}


def _get_program():
    if "nc" not in _PROGRAM_CACHE:
        _PROGRAM_CACHE["nc"] = build_program()
    return _PROGRAM_CACHE["nc"]


def host_prep(x, w0, b0, w1, b1, w2, b2):
    bf = ml_dtypes.bfloat16
    x = np.asarray(x, dtype=np.float32)

    # layer-0 banded-symmetric weight packing (rows (u,t) = pair (i, i+t))
    w0f = np.asarray(w0, np.float32).reshape(F0, F0, S)
    w0sym = w0f + np.transpose(w0f, (1, 0, 2))
    w0sb = np.zeros((117, NC0 * S), np.float32)
    for c, (i0, L, k) in enumerate(L0_CHUNKS):
        for u in range(k):
            i = i0 + u
            for tt in range(L):
                j = i + tt
                if i < F0 and j < F0:
                    w0sb[u * L + tt, S * c:S * (c + 1)] = (
                        w0f[i, j] if tt == 0 else w0sym[i, j])
    w0sb = np.ascontiguousarray(w0sb.astype(bf))

    w1f = np.asarray(w1, np.float32).reshape(F0, S, S)
    w1sb = np.ascontiguousarray(
        w1f.transpose(1, 0, 2).reshape(S, M1).astype(bf))
    w2f = np.asarray(w2, np.float32).reshape(F0, S, S)
    w2sb = np.ascontiguousarray(
        w2f.transpose(1, 0, 2).reshape(S, M1).astype(bf))

    b0v = np.ascontiguousarray(np.asarray(b0, np.float32).reshape(S, 1))
    b1v = np.ascontiguousarray(np.asarray(b1, np.float32).reshape(S, 1))
    b1x = np.ascontiguousarray(D * np.asarray(b1, np.float32).reshape(S, 1))
    b2x = np.ascontiguousarray(D * np.asarray(b2, np.float32).reshape(S, 1))

    # selector rows cover the full 128-partition height
    selp = np.zeros((F0, 128, S), np.float32)
    for i in range(F0):
        selp[i, i, :] = 1.0
    selp = np.ascontiguousarray(selp.transpose(1, 0, 2)
                                .reshape(128, F0 * S)).astype(bf)

    in_maps = []
    for cid in range(N_CORES):
        xs = x[BC * cid:BC * (cid + 1)]                  # [512, 39, 16]
        xTpv = np.zeros((XTP, BD), np.float32)
        xTpv[:F0] = xs.transpose(1, 0, 2).reshape(F0, BD)
        xTpv = np.ascontiguousarray(xTpv.astype(bf))
        # block-diagonal gram probe, host-built:
        # xbdh[16k+d, gl*GN + kk*F0 + i] = xs[8gl+kk, i, d] iff kk==k
        blk = xs.reshape(NGRP, GB, F0, D)                # [gl, kk, i, d]
        arr = np.zeros((GB, D, NGRP, GB, F0), np.float32)
        for k in range(GB):
            arr[k, :, :, k, :] = blk[:, k].transpose(2, 0, 1)
        xbdh = arr.reshape(128, NGRP * GN).astype(bf)
        xbdh0 = np.ascontiguousarray(xbdh[:, :NGRP * GN // 2])
        xbdh1 = np.ascontiguousarray(xbdh[:, NGRP * GN // 2:])
        in_maps.append({"xTp": xTpv, "w0sb": w0sb, "w1sb": w1sb,
                        "w2sb": w2sb, "xbdh0": xbdh0, "xbdh1": xbdh1,
                        "selp": selp,
                        "b0": b0v, "b1": b1v, "b1x": b1x, "b2x": b2x})
    return in_maps


def kernel(x, w0, b0, w1, b1, w2, b2):
    in_maps = host_prep(x, w0, b0, w1, b1, w2, b2)
    nc = _get_program()
    res = run_bass_kernel_spmd(nc, in_maps, core_ids=list(range(N_CORES)),
                               trace=False)
    return np.concatenate([r["out"] for r in res.results], axis=0)
